# revision 20
# baseline (speedup 1.0000x reference)
"""Trainium2 Bass kernel for the Engram scatter-memory module.

Strategy
--------
The reference runs a sequential scan over B=2048 samples. At step t it
computes cosine similarity of sample t against the current prototype bank,
then either REINFORCES the best match (if max_sim >= 0.7) or CREATES a new
prototype (append; the bank holds P=2048 slots and B <= P so it never
overflows and the "overwrite weakest" branch is unreachable). Feedback is
zero on create and (new_proto - p) * forca on reinforce.

Key observation: as long as no reinforce has happened up to step t, the bank
contains exactly samples 0..t-1. Therefore the scan decision at every step is
fully determined by the *causal cosine-similarity maxima*
    m[t] = max_{j < t} cos(p_t, p_j).
If m[t] < 0.7 for all t (by induction the bank then always holds the raw
prefix of samples), every step creates and the output is exactly zero.

The device kernel computes m[t] for all t on 8 NeuronCores:
  - Host normalizes rows (cheap, 0.05% of total FLOPs) and feeds the
    transposed normalized matrix NT [D=1024, B=2048] in bf16.
  - Core k owns the two 128-row tiles k and 15-k of the Gram matrix
    (pairing balances the causal-triangle work exactly: each core does
    the same matmul shape/count -> a single SPMD program).
  - Gram blocks are computed on the tensor engine (bf16 in, fp32 PSUM
    accumulation), additive -1e30 masks (supplied as per-core input data)
    enforce strict causality, and the vector engine reduces row maxima.
Host gathers m, and if max(m) < 0.45 (a guard band far below the 0.7
threshold: bf16 cosine error is ~1e-2 at most) certifies the no-reinforce
trajectory and returns the exact all-zero output. Otherwise an exact
numpy port of the reference scan runs as fallback.
"""

import numpy as np

B, D, P = 2048, 1024, 2048
NCORES = 8
NCHUNK = D // 128  # contraction chunks of 128

LIMIAR_SIM = 0.7
LIMIAR_NOVO = 0.3
LR = 0.01
EPS = 1e-8
GUARD = 0.45  # certify no-reinforce only if every causal max is below this
# Additive mask in fp8e4 (max finite 240): masked entries end at sim-240,
# far below any valid cosine sim (>= -1). Host treats < -200 as -inf.
MASK_VAL = -240.0

_PROGRAM = None
_LAST_RESULT = None  # BassKernelResults of the last device run (for test.py)


# --------------------------------------------------------------------------
# Device program: causal cosine-sim row maxima, one SPMD program for 8 cores
# --------------------------------------------------------------------------

def _build_program():
    import concourse.bacc as bacc
    import concourse.mybir as mybir
    from concourse.tile import TileContext

    nc = bacc.Bacc("TRN2", target_bir_lowering=False)
    fp8 = mybir.dt.float8e4
    f32 = mybir.dt.float32
    X = mybir.AxisListType.X

    # Replicated: normalized patterns, transposed [D, B].
    nt = nc.declare_dram_parameter("nt", [D, B], fp8, isOutput=False)
    # Per-core: stationary columns for row-tiles A (=k) and B (=15-k).
    rt = nc.declare_dram_parameter("rt", [D, 256], fp8, isOutput=False)
    # Per-core additive causality masks (0 valid / -240 invalid). They are
    # folded into the PSUM accumulation as an extra matmul eye^T @ mask so
    # the vector engine can reduce straight out of PSUM.
    maskA = nc.declare_dram_parameter("maskA", [128, 1024], fp8, isOutput=False)
    maskB = nc.declare_dram_parameter("maskB", [128, B], fp8, isOutput=False)
    eye = nc.declare_dram_parameter("eye", [128, 128], fp8, isOutput=False)
    # Row maxima for the two owned tiles.
    mout = nc.declare_dram_parameter("mout", [2, 128], f32, isOutput=True)

    # tiles per row-tile: A covers moving cols [0,1024), B covers [0,2048)
    NB = (2, 4)
    NGROUPS = sum(NB)  # 6 PSUM banks

    with TileContext(nc) as tc:
        with (
            tc.tile_pool(name="data", bufs=1) as data_pool,
            tc.tile_pool(name="red", bufs=1) as red_pool,
            tc.tile_pool(name="ps", bufs=1, space="PSUM") as ps_pool,
        ):
            # Per-chunk DMAs so the tensor engine starts as soon as the
            # first 128-row contraction chunk lands (DMA/PE pipelining).
            nt_sb = data_pool.tile([128, NCHUNK, B], fp8, tag="nt_sb")
            rt_sb = data_pool.tile([128, NCHUNK, 256], fp8, tag="rt_sb")
            mA_sb = data_pool.tile([128, 1024], fp8, tag="mA_sb")
            mB_sb = data_pool.tile([128, B], fp8, tag="mB_sb")
            eye_sb = data_pool.tile([128, 128], fp8, tag="eye_sb")
            nc.sync.dma_start(out=rt_sb[:, :, :],
                              in_=rt.rearrange("(c p) b -> p c b", p=128))
            nc.sync.dma_start(out=eye_sb[:, :], in_=eye[:, :])
            for c in range(NCHUNK):
                nc.sync.dma_start(out=nt_sb[:, c, :],
                                  in_=nt[c * 128:(c + 1) * 128, :])
            nc.sync.dma_start(out=mA_sb[:, :], in_=maskA[:, :])
            nc.sync.dma_start(out=mB_sb[:, :], in_=maskB[:, :])

            masks = (mA_sb, mB_sb)
            ps = [
                ps_pool.tile([128, 512], f32, tag=f"ps{g}", name=f"ps{g}")
                for g in range(NGROUPS)
            ]
            # Chunk-outer loop: every PSUM group accumulates chunk c right
            # after its DMA lands; PE never waits for the full load. The
            # group-0-last order in the final chunk plus per-group mask
            # matmuls lets the vector engine start reducing group 0 while
            # the PE finishes the others.
            for c in range(NCHUNK):
                g = 0
                for ti in range(2):
                    for jb in range(NB[ti]):
                        nc.tensor.matmul(
                            ps[g][:, :],
                            rt_sb[:, c, ti * 128:(ti + 1) * 128],
                            nt_sb[:, c, jb * 512:(jb + 1) * 512],
                            start=(c == 0),
                            stop=False,
                        )
                        g += 1
            # Fold the additive causality mask into the accumulation:
            # eye^T @ mask == mask, so PSUM ends as sims + mask.
            mcols = []
            g = 0
            for ti in range(2):
                nb = NB[ti]
                mcol = red_pool.tile([128, nb], f32, tag=f"mcol{ti}", name=f"mcol{ti}")
                mcols.append(mcol)
                for jb in range(nb):
                    nc.tensor.matmul(
                        ps[g][:, :],
                        eye_sb[:, :],
                        masks[ti][:, jb * 512:(jb + 1) * 512],
                        start=False,
                        stop=True,
                    )
                    nc.vector.reduce_max(
                        mcol[:, jb:jb + 1], ps[g][:, :], axis=X
                    )
                    g += 1
            for ti in range(2):
                mfin = red_pool.tile([128, 1], f32, tag=f"mfin{ti}", name=f"mfin{ti}")
                nc.vector.reduce_max(mfin[:, :], mcols[ti][:, :], axis=X)
                nc.sync.dma_start(out=mout[ti:ti + 1, :], in_=mfin[:, 0:1])

    nc.compile()
    return nc


def _prepare_in_maps(padrao_f32):
    import ml_dtypes

    fp8 = ml_dtypes.float8_e4m3

    norms = np.linalg.norm(padrao_f32, axis=1)
    norms = np.maximum(norms, EPS)
    nrm = padrao_f32 / norms[:, None]
    nt_fp8 = np.ascontiguousarray(nrm.T).astype(fp8)

    rows128 = np.arange(128)[:, None]
    cols1024 = np.arange(1024)[None, :]
    colsB = np.arange(B)[None, :]
    eye = np.eye(128, dtype=fp8)

    in_maps = []
    for k in range(NCORES):
        ta, tb = k * 128, (15 - k) * 128
        rt = np.ascontiguousarray(
            np.concatenate([nt_fp8[:, ta:ta + 128], nt_fp8[:, tb:tb + 128]], axis=1)
        )
        mask_a = np.where(cols1024 < ta + rows128, 0.0, MASK_VAL).astype(fp8)
        mask_b = np.where(colsB < tb + rows128, 0.0, MASK_VAL).astype(fp8)
        in_maps.append(
            {"nt": nt_fp8, "rt": rt, "maskA": mask_a, "maskB": mask_b, "eye": eye}
        )
    return in_maps


def _device_causal_maxes(padrao_f32, **run_kwargs):
    """Run the 8-core Bass kernel; returns m[t] = max_{j<t} cos(p_t, p_j)."""
    global _PROGRAM, _LAST_RESULT
    from concourse.bass_utils import run_bass_kernel_spmd

    if _PROGRAM is None:
        _PROGRAM = _build_program()
    in_maps = _prepare_in_maps(padrao_f32)
    _LAST_RESULT = run_bass_kernel_spmd(
        _PROGRAM, in_maps, list(range(NCORES)), **run_kwargs
    )
    m = np.empty(B, np.float32)
    for k in range(NCORES):
        mo = np.asarray(_LAST_RESULT.results[k]["mout"], dtype=np.float32)
        m[k * 128:(k + 1) * 128] = mo[0]
        m[(15 - k) * 128:(16 - k) * 128] = mo[1]
    return m


# --------------------------------------------------------------------------
# Exact scalar fallback (bit-faithful port of the reference scan)
# --------------------------------------------------------------------------

def _numpy_reference(padrao, erro, prototipos, forca, idade, n_protos):
    Pn, Dn = prototipos.shape
    protos = prototipos.astype(np.float32).copy()
    frc = forca.astype(np.float32).copy()
    age = idade.astype(np.int32).copy()
    n = int(n_protos)
    ar = np.arange(Pn)
    fbs = np.zeros_like(padrao, dtype=np.float32)
    for t in range(padrao.shape[0]):
        p = padrao[t]
        pn = max(np.linalg.norm(p), EPS)
        protn = np.maximum(np.linalg.norm(protos, axis=1), EPS)
        sim = (protos @ p) / (protn * pn)
        sim_m = np.where(ar < n, sim, -np.inf)
        idx = int(np.argmax(sim_m))
        max_sim = sim_m[idx]
        is_empty = n == 0
        do_reinf = (not is_empty) and (max_sim >= LIMIAR_SIM)
        do_create = is_empty or (
            (not do_reinf) and ((erro[t] > LIMIAR_NOVO) or (n < Pn))
        )
        if do_create:
            if n >= Pn:
                cidx = int(np.argmin(np.where(ar < n, frc, np.inf)))
            else:
                cidx = n
            protos[cidx] = p
            frc[cidx] = 1.0
            age[cidx] = 0
            if n < Pn:
                n += 1
            # feedback stays zero
        elif do_reinf:
            new_proto = (1.0 - LR) * protos[idx] + LR * p
            protos[idx] = new_proto
            frc[idx] += LR
            age[idx] += 1
            fbs[t] = (new_proto - p) * frc[idx]
        else:
            fbs[t] = (protos[idx] - p) * frc[idx]
    return fbs


# --------------------------------------------------------------------------
# Entry point
# --------------------------------------------------------------------------

def kernel(padrao, erro, prototipos, forca, idade, n_protos, feedback_gain):
    padrao = np.asarray(padrao, dtype=np.float32)
    erro = np.asarray(erro, dtype=np.float32)
    prototipos = np.asarray(prototipos, dtype=np.float32)
    forca = np.asarray(forca, dtype=np.float32)
    idade = np.asarray(idade, dtype=np.int32)
    n0 = int(np.asarray(n_protos))
    gain = np.float32(np.asarray(feedback_gain))

    fast_ok = (
        padrao.shape == (B, D)
        and prototipos.shape == (P, D)
        and n0 == 0
        and not prototipos.any()
        and not forca.any()
        and not idade.any()
        and B <= P
    )
    if fast_ok:
        m = _device_causal_maxes(padrao)
        # NaN-safe: NaN fails the comparison -> falls back.
        if np.max(m) < GUARD:
            # No step can reinforce (causal max far below 0.7 even after
            # bf16 error); bank starts empty and B <= P, so every step
            # creates and feedback is exactly zero.
            return np.zeros((B, D), np.float32)

    fbs = _numpy_reference(padrao, erro, prototipos, forca, idade, n0)
    return fbs * gain


# revision 23
# speedup vs baseline: 1.2314x; 1.2314x over previous
"""Trainium2 Bass kernel for the Engram scatter-memory module.

Strategy
--------
The reference runs a sequential scan over B=2048 samples. At step t it
computes cosine similarity of sample t against the current prototype bank,
then either REINFORCES the best match (if max_sim >= 0.7) or CREATES a new
prototype (append; the bank holds P=2048 slots and B <= P so it never
overflows and the "overwrite weakest" branch is unreachable). Feedback is
zero on create and (new_proto - p) * forca on reinforce.

Key observation: as long as no reinforce has happened up to step t, the bank
contains exactly samples 0..t-1. Therefore the scan decision at every step is
fully determined by the *causal cosine-similarity maxima*
    m[t] = max_{j < t} cos(p_t, p_j).
If m[t] < 0.7 for all t (by induction the bank then always holds the raw
prefix of samples), every step creates and the output is exactly zero.

The device kernel computes m[t] for all t on 8 NeuronCores:
  - Host normalizes rows (cheap, 0.05% of total FLOPs) and feeds the
    transposed normalized matrix NT [D=1024, B=2048] in bf16.
  - Core k owns the two 128-row tiles k and 15-k of the Gram matrix
    (pairing balances the causal-triangle work exactly: each core does
    the same matmul shape/count -> a single SPMD program).
  - Gram blocks are computed on the tensor engine (bf16 in, fp32 PSUM
    accumulation), additive -1e30 masks (supplied as per-core input data)
    enforce strict causality, and the vector engine reduces row maxima.
Host gathers m, and if max(m) < 0.45 (a guard band far below the 0.7
threshold: bf16 cosine error is ~1e-2 at most) certifies the no-reinforce
trajectory and returns the exact all-zero output. Otherwise an exact
numpy port of the reference scan runs as fallback.
"""

import numpy as np

B, D, P = 2048, 1024, 2048
NCORES = 8
NCHUNK = D // 128  # contraction chunks of 128

LIMIAR_SIM = 0.7
LIMIAR_NOVO = 0.3
LR = 0.01
EPS = 1e-8
GUARD = 0.45  # certify no-reinforce only if every causal max is below this
# Additive mask in fp8e4 (max finite 240): masked entries end at sim-240,
# far below any valid cosine sim (>= -1). Host treats < -200 as -inf.
MASK_VAL = -240.0

_PROGRAM = None
_LAST_RESULT = None  # BassKernelResults of the last device run (for test.py)


# --------------------------------------------------------------------------
# Device program: causal cosine-sim row maxima, one SPMD program for 8 cores
# --------------------------------------------------------------------------

def _build_program():
    import concourse.bacc as bacc
    import concourse.mybir as mybir
    from concourse.masks import make_identity
    from concourse.tile import TileContext

    nc = bacc.Bacc("TRN2", target_bir_lowering=False)
    fp8 = mybir.dt.float8e4
    f32 = mybir.dt.float32
    X = mybir.AxisListType.X

    # Replicated: normalized patterns, transposed [D, B].
    nt = nc.declare_dram_parameter("nt", [D, B], fp8, isOutput=False)
    # Per-core: stationary columns for row-tiles A (=k) and B (=15-k).
    rt = nc.declare_dram_parameter("rt", [D, 256], fp8, isOutput=False)
    # Per-core additive causality masks (0 valid / -240 invalid). They are
    # folded into the PSUM accumulation as an extra matmul eye^T @ mask so
    # the vector engine can reduce straight out of PSUM.
    maskA = nc.declare_dram_parameter("maskA", [128, 1024], fp8, isOutput=False)
    maskB = nc.declare_dram_parameter("maskB", [128, B], fp8, isOutput=False)
    eye = nc.declare_dram_parameter("eye", [128, 128], fp8, isOutput=False)
    # Row maxima for the two owned tiles.
    mout = nc.declare_dram_parameter("mout", [2, 128], f32, isOutput=True)

    # tiles per row-tile: A covers moving cols [0,1024), B covers [0,2048)
    NB = (2, 4)
    NGROUPS = sum(NB)  # 6 PSUM banks

    with TileContext(nc) as tc:
        with (
            tc.tile_pool(name="data", bufs=1) as data_pool,
            tc.tile_pool(name="red", bufs=1) as red_pool,
            tc.tile_pool(name="ps", bufs=1, space="PSUM") as ps_pool,
        ):
            # Per-chunk DMAs so the tensor engine starts as soon as the
            # first 128-row contraction chunk lands (DMA/PE pipelining).
            # Masks/eye load last: they are only needed after all chunks.
            nt_sb = data_pool.tile([128, NCHUNK, B], fp8, tag="nt_sb")
            rt_sb = data_pool.tile([128, NCHUNK, 256], fp8, tag="rt_sb")
            mA_sb = data_pool.tile([128, 1024], fp8, tag="mA_sb")
            mB_sb = data_pool.tile([128, B], fp8, tag="mB_sb")
            eye_sb = data_pool.tile([128, 128], fp8, tag="eye_sb")
            for c in range(NCHUNK):
                nc.sync.dma_start(out=rt_sb[:, c, :],
                                  in_=rt[c * 128:(c + 1) * 128, :])
                nc.sync.dma_start(out=nt_sb[:, c, :],
                                  in_=nt[c * 128:(c + 1) * 128, :])
            nc.sync.dma_start(out=eye_sb[:, :], in_=eye[:, :])
            nc.sync.dma_start(out=mA_sb[:, :], in_=maskA[:, :])
            nc.sync.dma_start(out=mB_sb[:, :], in_=maskB[:, :])
            # f32 identity for the PE-mode transpose of the result vector.
            ident = data_pool.tile([128, 128], f32, tag="ident")
            make_identity(nc, ident[:, :])

            masks = (mA_sb, mB_sb)
            ps = [
                ps_pool.tile([128, 512], f32, tag=f"ps{g}", name=f"ps{g}")
                for g in range(NGROUPS)
            ]
            # Chunk-outer loop: every PSUM group accumulates chunk c right
            # after its DMA lands; PE never waits for the full load. The
            # group-0-last order in the final chunk plus per-group mask
            # matmuls lets the vector engine start reducing group 0 while
            # the PE finishes the others.
            for c in range(NCHUNK):
                g = 0
                for ti in range(2):
                    for jb in range(NB[ti]):
                        nc.tensor.matmul(
                            ps[g][:, :],
                            rt_sb[:, c, ti * 128:(ti + 1) * 128],
                            nt_sb[:, c, jb * 512:(jb + 1) * 512],
                            start=(c == 0),
                            stop=False,
                        )
                        g += 1
            # Fold the additive causality mask into the accumulation:
            # eye^T @ mask == mask, so PSUM ends as sims + mask.
            mcols = []
            g = 0
            for ti in range(2):
                nb = NB[ti]
                mcol = red_pool.tile([128, nb], f32, tag=f"mcol{ti}", name=f"mcol{ti}")
                mcols.append(mcol)
                for jb in range(nb):
                    nc.tensor.matmul(
                        ps[g][:, :],
                        eye_sb[:, :],
                        masks[ti][:, jb * 512:(jb + 1) * 512],
                        start=False,
                        stop=True,
                    )
                    nc.vector.reduce_max(
                        mcol[:, jb:jb + 1], ps[g][:, :], axis=X
                    )
                    g += 1
            # Final per-row maxima land as [128 rows, 2 tiles]; a direct
            # [128,1]->[1,128] DMA degenerates to 128 4-byte descriptors
            # (~6 us!), so transpose on the PE first and store one
            # contiguous [2,128] block.
            mfin = red_pool.tile([128, 2], f32, tag="mfin")
            for ti in range(2):
                nc.vector.reduce_max(mfin[:, ti:ti + 1], mcols[ti][:, :], axis=X)
            ps_t = ps_pool.tile([2, 128], f32, tag="ps_t")
            nc.tensor.transpose(ps_t[:, :], mfin[:, :], ident[:, :])
            mrow = red_pool.tile([2, 128], f32, tag="mrow")
            nc.vector.tensor_copy(mrow[:, :], ps_t[:, :])
            nc.sync.dma_start(out=mout[:, :], in_=mrow[:, :])

    nc.compile()
    return nc


def _prepare_in_maps(padrao_f32):
    import ml_dtypes

    fp8 = ml_dtypes.float8_e4m3

    norms = np.linalg.norm(padrao_f32, axis=1)
    norms = np.maximum(norms, EPS)
    nrm = padrao_f32 / norms[:, None]
    nt_fp8 = np.ascontiguousarray(nrm.T).astype(fp8)

    rows128 = np.arange(128)[:, None]
    cols1024 = np.arange(1024)[None, :]
    colsB = np.arange(B)[None, :]
    eye = np.eye(128, dtype=fp8)

    in_maps = []
    for k in range(NCORES):
        ta, tb = k * 128, (15 - k) * 128
        rt = np.ascontiguousarray(
            np.concatenate([nt_fp8[:, ta:ta + 128], nt_fp8[:, tb:tb + 128]], axis=1)
        )
        mask_a = np.where(cols1024 < ta + rows128, 0.0, MASK_VAL).astype(fp8)
        mask_b = np.where(colsB < tb + rows128, 0.0, MASK_VAL).astype(fp8)
        in_maps.append(
            {"nt": nt_fp8, "rt": rt, "maskA": mask_a, "maskB": mask_b, "eye": eye}
        )
    return in_maps


def _device_causal_maxes(padrao_f32, **run_kwargs):
    """Run the 8-core Bass kernel; returns m[t] = max_{j<t} cos(p_t, p_j)."""
    global _PROGRAM, _LAST_RESULT
    from concourse.bass_utils import run_bass_kernel_spmd

    if _PROGRAM is None:
        _PROGRAM = _build_program()
    in_maps = _prepare_in_maps(padrao_f32)
    _LAST_RESULT = run_bass_kernel_spmd(
        _PROGRAM, in_maps, list(range(NCORES)), **run_kwargs
    )
    m = np.empty(B, np.float32)
    for k in range(NCORES):
        mo = np.asarray(_LAST_RESULT.results[k]["mout"], dtype=np.float32)
        m[k * 128:(k + 1) * 128] = mo[0]
        m[(15 - k) * 128:(16 - k) * 128] = mo[1]
    return m


# --------------------------------------------------------------------------
# Exact scalar fallback (bit-faithful port of the reference scan)
# --------------------------------------------------------------------------

def _numpy_reference(padrao, erro, prototipos, forca, idade, n_protos):
    Pn, Dn = prototipos.shape
    protos = prototipos.astype(np.float32).copy()
    frc = forca.astype(np.float32).copy()
    age = idade.astype(np.int32).copy()
    n = int(n_protos)
    ar = np.arange(Pn)
    fbs = np.zeros_like(padrao, dtype=np.float32)
    for t in range(padrao.shape[0]):
        p = padrao[t]
        pn = max(np.linalg.norm(p), EPS)
        protn = np.maximum(np.linalg.norm(protos, axis=1), EPS)
        sim = (protos @ p) / (protn * pn)
        sim_m = np.where(ar < n, sim, -np.inf)
        idx = int(np.argmax(sim_m))
        max_sim = sim_m[idx]
        is_empty = n == 0
        do_reinf = (not is_empty) and (max_sim >= LIMIAR_SIM)
        do_create = is_empty or (
            (not do_reinf) and ((erro[t] > LIMIAR_NOVO) or (n < Pn))
        )
        if do_create:
            if n >= Pn:
                cidx = int(np.argmin(np.where(ar < n, frc, np.inf)))
            else:
                cidx = n
            protos[cidx] = p
            frc[cidx] = 1.0
            age[cidx] = 0
            if n < Pn:
                n += 1
            # feedback stays zero
        elif do_reinf:
            new_proto = (1.0 - LR) * protos[idx] + LR * p
            protos[idx] = new_proto
            frc[idx] += LR
            age[idx] += 1
            fbs[t] = (new_proto - p) * frc[idx]
        else:
            fbs[t] = (protos[idx] - p) * frc[idx]
    return fbs


# --------------------------------------------------------------------------
# Entry point
# --------------------------------------------------------------------------

def kernel(padrao, erro, prototipos, forca, idade, n_protos, feedback_gain):
    padrao = np.asarray(padrao, dtype=np.float32)
    erro = np.asarray(erro, dtype=np.float32)
    prototipos = np.asarray(prototipos, dtype=np.float32)
    forca = np.asarray(forca, dtype=np.float32)
    idade = np.asarray(idade, dtype=np.int32)
    n0 = int(np.asarray(n_protos))
    gain = np.float32(np.asarray(feedback_gain))

    fast_ok = (
        padrao.shape == (B, D)
        and prototipos.shape == (P, D)
        and n0 == 0
        and not prototipos.any()
        and not forca.any()
        and not idade.any()
        and B <= P
    )
    if fast_ok:
        m = _device_causal_maxes(padrao)
        # NaN-safe: NaN fails the comparison -> falls back.
        if np.max(m) < GUARD:
            # No step can reinforce (causal max far below 0.7 even after
            # bf16 error); bank starts empty and B <= P, so every step
            # creates and feedback is exactly zero.
            return np.zeros((B, D), np.float32)

    fbs = _numpy_reference(padrao, erro, prototipos, forca, idade, n0)
    return fbs * gain


# revision 25
# speedup vs baseline: 1.3733x; 1.1152x over previous
"""Trainium2 Bass kernel for the Engram scatter-memory module.

Strategy
--------
The reference runs a sequential scan over B=2048 samples. At step t it
computes cosine similarity of sample t against the current prototype bank,
then either REINFORCES the best match (if max_sim >= 0.7) or CREATES a new
prototype (append; the bank holds P=2048 slots and B <= P so it never
overflows and the "overwrite weakest" branch is unreachable). Feedback is
zero on create and (new_proto - p) * forca on reinforce.

Key observation: as long as no reinforce has happened up to step t, the bank
contains exactly samples 0..t-1. Therefore the scan decision at every step is
fully determined by the *causal cosine-similarity maxima*
    m[t] = max_{j < t} cos(p_t, p_j).
If m[t] < 0.7 for all t (by induction the bank then always holds the raw
prefix of samples), every step creates and the output is exactly zero.

The device kernel computes m[t] for all t on 8 NeuronCores:
  - Host normalizes rows (cheap, 0.05% of total FLOPs) and feeds the
    transposed normalized matrix NT [D=1024, B=2048] in bf16.
  - Core k owns the two 128-row tiles k and 15-k of the Gram matrix
    (pairing balances the causal-triangle work exactly: each core does
    the same matmul shape/count -> a single SPMD program).
  - Gram blocks are computed on the tensor engine (bf16 in, fp32 PSUM
    accumulation), additive -1e30 masks (supplied as per-core input data)
    enforce strict causality, and the vector engine reduces row maxima.
Host gathers m, and if max(m) < 0.45 (a guard band far below the 0.7
threshold: bf16 cosine error is ~1e-2 at most) certifies the no-reinforce
trajectory and returns the exact all-zero output. Otherwise an exact
numpy port of the reference scan runs as fallback.
"""

import numpy as np

B, D, P = 2048, 1024, 2048
NCORES = 8
NCHUNK = D // 128  # contraction chunks of 128

LIMIAR_SIM = 0.7
LIMIAR_NOVO = 0.3
LR = 0.01
EPS = 1e-8
GUARD = 0.45  # certify no-reinforce only if every causal max is below this
# Additive mask in fp8e4 (max finite 240): masked entries end at sim-240,
# far below any valid cosine sim (>= -1). Host treats < -200 as -inf.
MASK_VAL = -240.0

_PROGRAM = None
_LAST_RESULT = None  # BassKernelResults of the last device run (for test.py)


# --------------------------------------------------------------------------
# Device program: causal cosine-sim row maxima, one SPMD program for 8 cores
# --------------------------------------------------------------------------

def _build_program():
    import concourse.bacc as bacc
    import concourse.mybir as mybir
    from concourse.masks import make_identity
    from concourse.tile import TileContext

    nc = bacc.Bacc("TRN2", target_bir_lowering=False)
    fp8 = mybir.dt.float8e4
    f32 = mybir.dt.float32
    X = mybir.AxisListType.X

    # Combined per-chunk input: for contraction chunk c (rows c*128..),
    # cols [0:256) are the stationary columns (row-tiles A=k, B=15-k) and
    # cols [256:2304) are all B moving columns. One DMA per chunk.
    data = nc.declare_dram_parameter("data", [D, 256 + B], fp8, isOutput=False)
    # Tail input: maskA (1024) | maskB (2048) | eye (128). The additive
    # causality masks (0 valid / -240 invalid) are folded into the PSUM
    # accumulation as an extra matmul eye^T @ mask so the vector engine
    # reduces straight out of PSUM.
    tail = nc.declare_dram_parameter("tail", [128, 3072 + 128], fp8, isOutput=False)
    # Row maxima for the two owned tiles.
    mout = nc.declare_dram_parameter("mout", [2, 128], f32, isOutput=True)

    W = 256 + B
    # tiles per row-tile: A covers moving cols [0,1024), B covers [0,2048)
    NB = (2, 4)
    NGROUPS = sum(NB)  # 6 PSUM banks

    with TileContext(nc) as tc:
        with (
            tc.tile_pool(name="data", bufs=1) as data_pool,
            tc.tile_pool(name="red", bufs=1) as red_pool,
            tc.tile_pool(name="ps", bufs=1, space="PSUM") as ps_pool,
        ):
            # Per-chunk DMAs so the tensor engine starts as soon as the
            # first 128-row contraction chunk lands (DMA/PE pipelining).
            # The mask/eye tail loads last: it is only needed at the end.
            data_sb = data_pool.tile([128, NCHUNK, W], fp8, tag="data_sb")
            tail_sb = data_pool.tile([128, 3200], fp8, tag="tail_sb")
            for c in range(NCHUNK):
                nc.sync.dma_start(out=data_sb[:, c, :],
                                  in_=data[c * 128:(c + 1) * 128, :])
            nc.sync.dma_start(out=tail_sb[:, :], in_=tail[:, :])
            # f32 identity for the PE-mode transpose of the result vector.
            ident = data_pool.tile([128, 128], f32, tag="ident")
            make_identity(nc, ident[:, :])

            eye_sb = tail_sb[:, 3072:3200]
            mask_off = (0, 1024)  # maskA at 0, maskB at 1024 within tail
            ps = [
                ps_pool.tile([128, 512], f32, tag=f"ps{g}", name=f"ps{g}")
                for g in range(NGROUPS)
            ]
            mcols = []
            # K-contiguous per group: all 8 contraction chunks back-to-back
            # into one PSUM bank (no bank cycling -> PE stays warm), then
            # the mask matmul closes the group and its reduction overlaps
            # the next groups' matmuls on the vector engine.
            g = 0
            for ti in range(2):
                nb = NB[ti]
                mcol = red_pool.tile([128, nb], f32, tag=f"mcol{ti}", name=f"mcol{ti}")
                mcols.append(mcol)
                for jb in range(nb):
                    for c in range(NCHUNK):
                        nc.tensor.matmul(
                            ps[g][:, :],
                            data_sb[:, c, ti * 128:(ti + 1) * 128],
                            data_sb[:, c, 256 + jb * 512:256 + (jb + 1) * 512],
                            start=(c == 0),
                            stop=False,
                        )
                    nc.tensor.matmul(
                        ps[g][:, :],
                        eye_sb,
                        tail_sb[:, mask_off[ti] + jb * 512:
                                mask_off[ti] + (jb + 1) * 512],
                        start=False,
                        stop=True,
                    )
                    nc.vector.reduce_max(
                        mcol[:, jb:jb + 1], ps[g][:, :], axis=X
                    )
                    g += 1
            # Final per-row maxima land as [128 rows, 2 tiles]; a direct
            # [128,1]->[1,128] DMA degenerates to 128 4-byte descriptors
            # (~6 us!), so transpose on the PE first and store one
            # contiguous [2,128] block.
            mfin = red_pool.tile([128, 2], f32, tag="mfin")
            for ti in range(2):
                nc.vector.reduce_max(mfin[:, ti:ti + 1], mcols[ti][:, :], axis=X)
            ps_t = ps_pool.tile([2, 128], f32, tag="ps_t")
            nc.tensor.transpose(ps_t[:, :], mfin[:, :], ident[:, :])
            mrow = red_pool.tile([2, 128], f32, tag="mrow")
            nc.vector.tensor_copy(mrow[:, :], ps_t[:, :])
            nc.sync.dma_start(out=mout[:, :], in_=mrow[:, :])

    nc.compile()
    return nc


def _prepare_in_maps(padrao_f32):
    import ml_dtypes

    fp8 = ml_dtypes.float8_e4m3

    norms = np.linalg.norm(padrao_f32, axis=1)
    norms = np.maximum(norms, EPS)
    nrm = padrao_f32 / norms[:, None]
    nt_fp8 = np.ascontiguousarray(nrm.T).astype(fp8)

    rows128 = np.arange(128)[:, None]
    cols1024 = np.arange(1024)[None, :]
    colsB = np.arange(B)[None, :]
    eye = np.eye(128, dtype=fp8)

    in_maps = []
    for k in range(NCORES):
        ta, tb = k * 128, (15 - k) * 128
        data = np.concatenate(
            [nt_fp8[:, ta:ta + 128], nt_fp8[:, tb:tb + 128], nt_fp8], axis=1
        )
        mask_a = np.where(cols1024 < ta + rows128, 0.0, MASK_VAL).astype(fp8)
        mask_b = np.where(colsB < tb + rows128, 0.0, MASK_VAL).astype(fp8)
        tail = np.concatenate([mask_a, mask_b, eye], axis=1)
        in_maps.append(
            {"data": np.ascontiguousarray(data), "tail": np.ascontiguousarray(tail)}
        )
    return in_maps


def _device_causal_maxes(padrao_f32, **run_kwargs):
    """Run the 8-core Bass kernel; returns m[t] = max_{j<t} cos(p_t, p_j)."""
    global _PROGRAM, _LAST_RESULT
    from concourse.bass_utils import run_bass_kernel_spmd

    if _PROGRAM is None:
        _PROGRAM = _build_program()
    in_maps = _prepare_in_maps(padrao_f32)
    _LAST_RESULT = run_bass_kernel_spmd(
        _PROGRAM, in_maps, list(range(NCORES)), **run_kwargs
    )
    m = np.empty(B, np.float32)
    for k in range(NCORES):
        mo = np.asarray(_LAST_RESULT.results[k]["mout"], dtype=np.float32)
        m[k * 128:(k + 1) * 128] = mo[0]
        m[(15 - k) * 128:(16 - k) * 128] = mo[1]
    return m


# --------------------------------------------------------------------------
# Exact scalar fallback (bit-faithful port of the reference scan)
# --------------------------------------------------------------------------

def _numpy_reference(padrao, erro, prototipos, forca, idade, n_protos):
    Pn, Dn = prototipos.shape
    protos = prototipos.astype(np.float32).copy()
    frc = forca.astype(np.float32).copy()
    age = idade.astype(np.int32).copy()
    n = int(n_protos)
    ar = np.arange(Pn)
    fbs = np.zeros_like(padrao, dtype=np.float32)
    for t in range(padrao.shape[0]):
        p = padrao[t]
        pn = max(np.linalg.norm(p), EPS)
        protn = np.maximum(np.linalg.norm(protos, axis=1), EPS)
        sim = (protos @ p) / (protn * pn)
        sim_m = np.where(ar < n, sim, -np.inf)
        idx = int(np.argmax(sim_m))
        max_sim = sim_m[idx]
        is_empty = n == 0
        do_reinf = (not is_empty) and (max_sim >= LIMIAR_SIM)
        do_create = is_empty or (
            (not do_reinf) and ((erro[t] > LIMIAR_NOVO) or (n < Pn))
        )
        if do_create:
            if n >= Pn:
                cidx = int(np.argmin(np.where(ar < n, frc, np.inf)))
            else:
                cidx = n
            protos[cidx] = p
            frc[cidx] = 1.0
            age[cidx] = 0
            if n < Pn:
                n += 1
            # feedback stays zero
        elif do_reinf:
            new_proto = (1.0 - LR) * protos[idx] + LR * p
            protos[idx] = new_proto
            frc[idx] += LR
            age[idx] += 1
            fbs[t] = (new_proto - p) * frc[idx]
        else:
            fbs[t] = (protos[idx] - p) * frc[idx]
    return fbs


# --------------------------------------------------------------------------
# Entry point
# --------------------------------------------------------------------------

def kernel(padrao, erro, prototipos, forca, idade, n_protos, feedback_gain):
    padrao = np.asarray(padrao, dtype=np.float32)
    erro = np.asarray(erro, dtype=np.float32)
    prototipos = np.asarray(prototipos, dtype=np.float32)
    forca = np.asarray(forca, dtype=np.float32)
    idade = np.asarray(idade, dtype=np.int32)
    n0 = int(np.asarray(n_protos))
    gain = np.float32(np.asarray(feedback_gain))

    fast_ok = (
        padrao.shape == (B, D)
        and prototipos.shape == (P, D)
        and n0 == 0
        and not prototipos.any()
        and not forca.any()
        and not idade.any()
        and B <= P
    )
    if fast_ok:
        m = _device_causal_maxes(padrao)
        # NaN-safe: NaN fails the comparison -> falls back.
        if np.max(m) < GUARD:
            # No step can reinforce (causal max far below 0.7 even after
            # bf16 error); bank starts empty and B <= P, so every step
            # creates and feedback is exactly zero.
            return np.zeros((B, D), np.float32)

    fbs = _numpy_reference(padrao, erro, prototipos, forca, idade, n0)
    return fbs * gain


# revision 26
# speedup vs baseline: 1.4061x; 1.0238x over previous
"""Trainium2 Bass kernel for the Engram scatter-memory module.

Strategy
--------
The reference runs a sequential scan over B=2048 samples. At step t it
computes cosine similarity of sample t against the current prototype bank,
then either REINFORCES the best match (if max_sim >= 0.7) or CREATES a new
prototype (append; the bank holds P=2048 slots and B <= P so it never
overflows and the "overwrite weakest" branch is unreachable). Feedback is
zero on create and (new_proto - p) * forca on reinforce.

Key observation: as long as no reinforce has happened up to step t, the bank
contains exactly samples 0..t-1. Therefore the scan decision at every step is
fully determined by the *causal cosine-similarity maxima*
    m[t] = max_{j < t} cos(p_t, p_j).
If m[t] < 0.7 for all t (by induction the bank then always holds the raw
prefix of samples), every step creates and the output is exactly zero.

The device kernel computes m[t] for all t on 8 NeuronCores:
  - Host normalizes rows (cheap, 0.05% of total FLOPs) and feeds the
    transposed normalized matrix NT [D=1024, B=2048] in bf16.
  - Core k owns the two 128-row tiles k and 15-k of the Gram matrix
    (pairing balances the causal-triangle work exactly: each core does
    the same matmul shape/count -> a single SPMD program).
  - Gram blocks are computed on the tensor engine (bf16 in, fp32 PSUM
    accumulation), additive -1e30 masks (supplied as per-core input data)
    enforce strict causality, and the vector engine reduces row maxima.
Host gathers m, and if max(m) < 0.45 (a guard band far below the 0.7
threshold: bf16 cosine error is ~1e-2 at most) certifies the no-reinforce
trajectory and returns the exact all-zero output. Otherwise an exact
numpy port of the reference scan runs as fallback.
"""

import numpy as np

B, D, P = 2048, 1024, 2048
NCORES = 8
NCHUNK = D // 128  # contraction chunks of 128

LIMIAR_SIM = 0.7
LIMIAR_NOVO = 0.3
LR = 0.01
EPS = 1e-8
GUARD = 0.45  # certify no-reinforce only if every causal max is below this
# Additive mask in fp8e4 (max finite 240): masked entries end at sim-240,
# far below any valid cosine sim (>= -1). Host treats < -200 as -inf.
MASK_VAL = -240.0

_PROGRAM = None
_LAST_RESULT = None  # BassKernelResults of the last device run (for test.py)


# --------------------------------------------------------------------------
# Device program: causal cosine-sim row maxima, one SPMD program for 8 cores
# --------------------------------------------------------------------------

def _build_program():
    import concourse.bacc as bacc
    import concourse.mybir as mybir
    from concourse.masks import make_identity
    from concourse.tile import TileContext

    nc = bacc.Bacc("TRN2", target_bir_lowering=False)
    fp8 = mybir.dt.float8e4
    f32 = mybir.dt.float32
    X = mybir.AxisListType.X

    # Combined per-chunk input: for contraction chunk c (rows c*128..),
    # cols [0:256) are the stationary columns (row-tiles A=k, B=15-k) and
    # cols [256:2304) are all B moving columns. One DMA per chunk.
    data = nc.declare_dram_parameter("data", [D, 256 + B], fp8, isOutput=False)
    # Tail input: maskA (1024) | maskB (2048) | eye (128). The additive
    # causality masks (0 valid / -240 invalid) are folded into the PSUM
    # accumulation as an extra matmul eye^T @ mask so the vector engine
    # reduces straight out of PSUM.
    tail = nc.declare_dram_parameter("tail", [128, 3072 + 128], fp8, isOutput=False)
    # Row maxima for the two owned tiles.
    mout = nc.declare_dram_parameter("mout", [2, 128], f32, isOutput=True)

    W = 256 + B
    # tiles per row-tile: A covers moving cols [0,1024), B covers [0,2048)
    NB = (2, 4)
    NGROUPS = sum(NB)  # 6 PSUM banks

    with TileContext(nc) as tc:
        with (
            tc.tile_pool(name="data", bufs=1) as data_pool,
            tc.tile_pool(name="red", bufs=1) as red_pool,
            tc.tile_pool(name="ps", bufs=1, space="PSUM") as ps_pool,
        ):
            # Per-chunk DMAs so the tensor engine starts as soon as the
            # first 128-row contraction chunk lands (DMA/PE pipelining).
            # The mask/eye tail loads last: it is only needed at the end.
            data_sb = data_pool.tile([128, NCHUNK, W], fp8, tag="data_sb")
            tail_sb = data_pool.tile([128, 3200], fp8, tag="tail_sb")
            for c in range(NCHUNK):
                nc.sync.dma_start(out=data_sb[:, c, :],
                                  in_=data[c * 128:(c + 1) * 128, :])
            nc.sync.dma_start(out=tail_sb[:, :], in_=tail[:, :])
            # f32 identity for the PE-mode transpose of the result vector.
            ident = data_pool.tile([128, 128], f32, tag="ident")
            make_identity(nc, ident[:, :])

            eye_sb = tail_sb[:, 3072:3200]
            mask_off = (0, 1024)  # maskA at 0, maskB at 1024 within tail
            ps = [
                ps_pool.tile([128, 512], f32, tag=f"ps{g}", name=f"ps{g}")
                for g in range(NGROUPS)
            ]
            mcols = []
            # K-contiguous per group: all 8 contraction chunks back-to-back
            # into one PSUM bank (no bank cycling -> PE stays warm), then
            # the mask matmul closes the group and its reduction overlaps
            # the next groups' matmuls on the vector engine.
            g = 0
            for ti in range(2):
                nb = NB[ti]
                mcol = red_pool.tile([128, nb], f32, tag=f"mcol{ti}", name=f"mcol{ti}")
                mcols.append(mcol)
                for jb in range(nb):
                    # DoubleRow packs two fp8 contraction chunks per matmul
                    # (virtual 128x256 array): half the MM+LDW count.
                    for cc in range(NCHUNK // 2):
                        nc.tensor.matmul(
                            ps[g][:, :],
                            data_sb[:, 2 * cc:2 * cc + 2, ti * 128:(ti + 1) * 128],
                            data_sb[:, 2 * cc:2 * cc + 2,
                                    256 + jb * 512:256 + (jb + 1) * 512],
                            start=(cc == 0),
                            stop=False,
                            perf_mode=mybir.MatmulPerfMode.DoubleRow,
                        )
                    nc.tensor.matmul(
                        ps[g][:, :],
                        eye_sb,
                        tail_sb[:, mask_off[ti] + jb * 512:
                                mask_off[ti] + (jb + 1) * 512],
                        start=False,
                        stop=True,
                    )
                    nc.vector.reduce_max(
                        mcol[:, jb:jb + 1], ps[g][:, :], axis=X
                    )
                    g += 1
            # Final per-row maxima land as [128 rows, 2 tiles]; a direct
            # [128,1]->[1,128] DMA degenerates to 128 4-byte descriptors
            # (~6 us!), so transpose on the PE first and store one
            # contiguous [2,128] block.
            mfin = red_pool.tile([128, 2], f32, tag="mfin")
            for ti in range(2):
                nc.vector.reduce_max(mfin[:, ti:ti + 1], mcols[ti][:, :], axis=X)
            ps_t = ps_pool.tile([2, 128], f32, tag="ps_t")
            nc.tensor.transpose(ps_t[:, :], mfin[:, :], ident[:, :])
            mrow = red_pool.tile([2, 128], f32, tag="mrow")
            nc.vector.tensor_copy(mrow[:, :], ps_t[:, :])
            nc.sync.dma_start(out=mout[:, :], in_=mrow[:, :])

    nc.compile()
    return nc


def _prepare_in_maps(padrao_f32):
    import ml_dtypes

    fp8 = ml_dtypes.float8_e4m3

    norms = np.linalg.norm(padrao_f32, axis=1)
    norms = np.maximum(norms, EPS)
    nrm = padrao_f32 / norms[:, None]
    nt_fp8 = np.ascontiguousarray(nrm.T).astype(fp8)

    rows128 = np.arange(128)[:, None]
    cols1024 = np.arange(1024)[None, :]
    colsB = np.arange(B)[None, :]
    eye = np.eye(128, dtype=fp8)

    in_maps = []
    for k in range(NCORES):
        ta, tb = k * 128, (15 - k) * 128
        data = np.concatenate(
            [nt_fp8[:, ta:ta + 128], nt_fp8[:, tb:tb + 128], nt_fp8], axis=1
        )
        mask_a = np.where(cols1024 < ta + rows128, 0.0, MASK_VAL).astype(fp8)
        mask_b = np.where(colsB < tb + rows128, 0.0, MASK_VAL).astype(fp8)
        tail = np.concatenate([mask_a, mask_b, eye], axis=1)
        in_maps.append(
            {"data": np.ascontiguousarray(data), "tail": np.ascontiguousarray(tail)}
        )
    return in_maps


def _device_causal_maxes(padrao_f32, **run_kwargs):
    """Run the 8-core Bass kernel; returns m[t] = max_{j<t} cos(p_t, p_j)."""
    global _PROGRAM, _LAST_RESULT
    from concourse.bass_utils import run_bass_kernel_spmd

    if _PROGRAM is None:
        _PROGRAM = _build_program()
    in_maps = _prepare_in_maps(padrao_f32)
    _LAST_RESULT = run_bass_kernel_spmd(
        _PROGRAM, in_maps, list(range(NCORES)), **run_kwargs
    )
    m = np.empty(B, np.float32)
    for k in range(NCORES):
        mo = np.asarray(_LAST_RESULT.results[k]["mout"], dtype=np.float32)
        m[k * 128:(k + 1) * 128] = mo[0]
        m[(15 - k) * 128:(16 - k) * 128] = mo[1]
    return m


# --------------------------------------------------------------------------
# Exact scalar fallback (bit-faithful port of the reference scan)
# --------------------------------------------------------------------------

def _numpy_reference(padrao, erro, prototipos, forca, idade, n_protos):
    Pn, Dn = prototipos.shape
    protos = prototipos.astype(np.float32).copy()
    frc = forca.astype(np.float32).copy()
    age = idade.astype(np.int32).copy()
    n = int(n_protos)
    ar = np.arange(Pn)
    fbs = np.zeros_like(padrao, dtype=np.float32)
    for t in range(padrao.shape[0]):
        p = padrao[t]
        pn = max(np.linalg.norm(p), EPS)
        protn = np.maximum(np.linalg.norm(protos, axis=1), EPS)
        sim = (protos @ p) / (protn * pn)
        sim_m = np.where(ar < n, sim, -np.inf)
        idx = int(np.argmax(sim_m))
        max_sim = sim_m[idx]
        is_empty = n == 0
        do_reinf = (not is_empty) and (max_sim >= LIMIAR_SIM)
        do_create = is_empty or (
            (not do_reinf) and ((erro[t] > LIMIAR_NOVO) or (n < Pn))
        )
        if do_create:
            if n >= Pn:
                cidx = int(np.argmin(np.where(ar < n, frc, np.inf)))
            else:
                cidx = n
            protos[cidx] = p
            frc[cidx] = 1.0
            age[cidx] = 0
            if n < Pn:
                n += 1
            # feedback stays zero
        elif do_reinf:
            new_proto = (1.0 - LR) * protos[idx] + LR * p
            protos[idx] = new_proto
            frc[idx] += LR
            age[idx] += 1
            fbs[t] = (new_proto - p) * frc[idx]
        else:
            fbs[t] = (protos[idx] - p) * frc[idx]
    return fbs


# --------------------------------------------------------------------------
# Entry point
# --------------------------------------------------------------------------

def kernel(padrao, erro, prototipos, forca, idade, n_protos, feedback_gain):
    padrao = np.asarray(padrao, dtype=np.float32)
    erro = np.asarray(erro, dtype=np.float32)
    prototipos = np.asarray(prototipos, dtype=np.float32)
    forca = np.asarray(forca, dtype=np.float32)
    idade = np.asarray(idade, dtype=np.int32)
    n0 = int(np.asarray(n_protos))
    gain = np.float32(np.asarray(feedback_gain))

    fast_ok = (
        padrao.shape == (B, D)
        and prototipos.shape == (P, D)
        and n0 == 0
        and not prototipos.any()
        and not forca.any()
        and not idade.any()
        and B <= P
    )
    if fast_ok:
        m = _device_causal_maxes(padrao)
        # NaN-safe: NaN fails the comparison -> falls back.
        if np.max(m) < GUARD:
            # No step can reinforce (causal max far below 0.7 even after
            # bf16 error); bank starts empty and B <= P, so every step
            # creates and feedback is exactly zero.
            return np.zeros((B, D), np.float32)

    fbs = _numpy_reference(padrao, erro, prototipos, forca, idade, n0)
    return fbs * gain


# revision 27
# speedup vs baseline: 1.4800x; 1.0526x over previous
"""Trainium2 Bass kernel for the Engram scatter-memory module.

Strategy
--------
The reference runs a sequential scan over B=2048 samples. At step t it
computes cosine similarity of sample t against the current prototype bank,
then either REINFORCES the best match (if max_sim >= 0.7) or CREATES a new
prototype (append; the bank holds P=2048 slots and B <= P so it never
overflows and the "overwrite weakest" branch is unreachable). Feedback is
zero on create and (new_proto - p) * forca on reinforce.

Key observation: as long as no reinforce has happened up to step t, the bank
contains exactly samples 0..t-1. Therefore the scan decision at every step is
fully determined by the *causal cosine-similarity maxima*
    m[t] = max_{j < t} cos(p_t, p_j).
If m[t] < 0.7 for all t (by induction the bank then always holds the raw
prefix of samples), every step creates and the output is exactly zero.

The device kernel computes m[t] for all t on 8 NeuronCores:
  - Host normalizes rows (cheap, 0.05% of total FLOPs) and feeds the
    transposed normalized matrix NT [D=1024, B=2048] in bf16.
  - Core k owns the two 128-row tiles k and 15-k of the Gram matrix
    (pairing balances the causal-triangle work exactly: each core does
    the same matmul shape/count -> a single SPMD program).
  - Gram blocks are computed on the tensor engine (bf16 in, fp32 PSUM
    accumulation), additive -1e30 masks (supplied as per-core input data)
    enforce strict causality, and the vector engine reduces row maxima.
Host gathers m, and if max(m) < 0.45 (a guard band far below the 0.7
threshold: bf16 cosine error is ~1e-2 at most) certifies the no-reinforce
trajectory and returns the exact all-zero output. Otherwise an exact
numpy port of the reference scan runs as fallback.
"""

import numpy as np

B, D, P = 2048, 1024, 2048
NCORES = 8
NCHUNK = D // 128  # contraction chunks of 128

LIMIAR_SIM = 0.7
LIMIAR_NOVO = 0.3
LR = 0.01
EPS = 1e-8
GUARD = 0.45  # certify no-reinforce only if every causal max is below this
# Additive mask in fp8e4 (max finite 240): masked entries end at sim-240,
# far below any valid cosine sim (>= -1). Host treats < -200 as -inf.
MASK_VAL = -240.0

_PROGRAM = None
_LAST_RESULT = None  # BassKernelResults of the last device run (for test.py)


# --------------------------------------------------------------------------
# Device program: causal cosine-sim row maxima, one SPMD program for 8 cores
# --------------------------------------------------------------------------

def _build_program():
    import concourse.bacc as bacc
    import concourse.mybir as mybir
    from concourse.masks import make_identity
    from concourse.tile import TileContext

    nc = bacc.Bacc("TRN2", target_bir_lowering=False)
    fp8 = mybir.dt.float8e4
    f32 = mybir.dt.float32
    X = mybir.AxisListType.X

    # Combined per-chunk input: for contraction chunk c (rows c*128..),
    # cols [0:256) are the stationary columns (row-tiles A=k, B=15-k) and
    # cols [256:2304) are all B moving columns. One DMA per chunk.
    data = nc.declare_dram_parameter("data", [D, 256 + B], fp8, isOutput=False)
    # Tail input: maskA (1024) | maskB (2048) | eye (128). The additive
    # causality masks (0 valid / -240 invalid) are folded into the PSUM
    # accumulation as an extra matmul eye^T @ mask so the vector engine
    # reduces straight out of PSUM.
    tail = nc.declare_dram_parameter("tail", [128, 3072 + 128], fp8, isOutput=False)
    # Row maxima for the two owned tiles.
    mout = nc.declare_dram_parameter("mout", [2, 128], f32, isOutput=True)

    W = 256 + B
    # tiles per row-tile: A covers moving cols [0,1024), B covers [0,2048)
    NB = (2, 4)
    NGROUPS = sum(NB)  # 6 PSUM banks

    with TileContext(nc) as tc:
        with (
            tc.tile_pool(name="data", bufs=1) as data_pool,
            tc.tile_pool(name="red", bufs=1) as red_pool,
            tc.tile_pool(name="ps", bufs=1, space="PSUM") as ps_pool,
        ):
            # Per-chunk DMAs so the tensor engine starts as soon as the
            # first 128-row contraction chunk lands (DMA/PE pipelining).
            # The mask/eye tail loads last: it is only needed at the end.
            data_sb = data_pool.tile([128, NCHUNK, W], fp8, tag="data_sb")
            tail_sb = data_pool.tile([128, 3200], fp8, tag="tail_sb")
            for c in range(3):
                nc.sync.dma_start(out=data_sb[:, c, :],
                                  in_=data[c * 128:(c + 1) * 128, :])
            # Masks/eye early enough that group 0's mask matmul (and thus
            # the whole reduction chain) isn't serialized to the end.
            nc.sync.dma_start(out=tail_sb[:, :], in_=tail[:, :])
            for c in range(3, NCHUNK):
                nc.sync.dma_start(out=data_sb[:, c, :],
                                  in_=data[c * 128:(c + 1) * 128, :])
            # f32 identity for the PE-mode transpose of the result vector.
            ident = data_pool.tile([128, 128], f32, tag="ident")
            make_identity(nc, ident[:, :])

            # The PE sits idle for ~6 us while the first chunks stream in,
            # which parks the HAM clock-gate at 1.2 GHz and makes the first
            # ~10 real matmuls run at half speed. Warm it up on a zeroed
            # scratch tile (results go to a dead PSUM bank, never read).
            warm_sb = data_pool.tile([128, 640], fp8, tag="warm_sb")
            nc.gpsimd.memset(warm_sb[:, :], 0.0)
            ps_warm = ps_pool.tile([128, 512], f32, tag="ps_warm")
            for _ in range(10):
                nc.tensor.matmul(
                    ps_warm[:, :], warm_sb[:, 0:128], warm_sb[:, 128:640],
                    start=True, stop=True,
                )

            eye_sb = tail_sb[:, 3072:3200]
            mask_off = (0, 1024)  # maskA at 0, maskB at 1024 within tail
            ps = [
                ps_pool.tile([128, 512], f32, tag=f"ps{g}", name=f"ps{g}")
                for g in range(NGROUPS)
            ]
            mcols = []
            # K-contiguous per group: all 8 contraction chunks back-to-back
            # into one PSUM bank (no bank cycling -> PE stays warm), then
            # the mask matmul closes the group and its reduction overlaps
            # the next groups' matmuls on the vector engine.
            g = 0
            for ti in range(2):
                nb = NB[ti]
                mcol = red_pool.tile([128, nb], f32, tag=f"mcol{ti}", name=f"mcol{ti}")
                mcols.append(mcol)
                for jb in range(nb):
                    # DoubleRow packs two fp8 contraction chunks per matmul
                    # (virtual 128x256 array): half the MM+LDW count.
                    for cc in range(NCHUNK // 2):
                        nc.tensor.matmul(
                            ps[g][:, :],
                            data_sb[:, 2 * cc:2 * cc + 2, ti * 128:(ti + 1) * 128],
                            data_sb[:, 2 * cc:2 * cc + 2,
                                    256 + jb * 512:256 + (jb + 1) * 512],
                            start=(cc == 0),
                            stop=False,
                            perf_mode=mybir.MatmulPerfMode.DoubleRow,
                        )
                    nc.tensor.matmul(
                        ps[g][:, :],
                        eye_sb,
                        tail_sb[:, mask_off[ti] + jb * 512:
                                mask_off[ti] + (jb + 1) * 512],
                        start=False,
                        stop=True,
                    )
                    nc.vector.reduce_max(
                        mcol[:, jb:jb + 1], ps[g][:, :], axis=X
                    )
                    g += 1
            # Final per-row maxima land as [128 rows, 2 tiles]; a direct
            # [128,1]->[1,128] DMA degenerates to 128 4-byte descriptors
            # (~6 us!), so transpose on the PE first and store one
            # contiguous [2,128] block.
            mfin = red_pool.tile([128, 2], f32, tag="mfin")
            for ti in range(2):
                nc.vector.reduce_max(mfin[:, ti:ti + 1], mcols[ti][:, :], axis=X)
            ps_t = ps_pool.tile([2, 128], f32, tag="ps_t")
            nc.tensor.transpose(ps_t[:, :], mfin[:, :], ident[:, :])
            mrow = red_pool.tile([2, 128], f32, tag="mrow")
            nc.vector.tensor_copy(mrow[:, :], ps_t[:, :])
            nc.sync.dma_start(out=mout[:, :], in_=mrow[:, :])

    nc.compile()
    return nc


def _prepare_in_maps(padrao_f32):
    import ml_dtypes

    fp8 = ml_dtypes.float8_e4m3

    norms = np.linalg.norm(padrao_f32, axis=1)
    norms = np.maximum(norms, EPS)
    nrm = padrao_f32 / norms[:, None]
    nt_fp8 = np.ascontiguousarray(nrm.T).astype(fp8)

    rows128 = np.arange(128)[:, None]
    cols1024 = np.arange(1024)[None, :]
    colsB = np.arange(B)[None, :]
    eye = np.eye(128, dtype=fp8)

    in_maps = []
    for k in range(NCORES):
        ta, tb = k * 128, (15 - k) * 128
        data = np.concatenate(
            [nt_fp8[:, ta:ta + 128], nt_fp8[:, tb:tb + 128], nt_fp8], axis=1
        )
        mask_a = np.where(cols1024 < ta + rows128, 0.0, MASK_VAL).astype(fp8)
        mask_b = np.where(colsB < tb + rows128, 0.0, MASK_VAL).astype(fp8)
        tail = np.concatenate([mask_a, mask_b, eye], axis=1)
        in_maps.append(
            {"data": np.ascontiguousarray(data), "tail": np.ascontiguousarray(tail)}
        )
    return in_maps


def _device_causal_maxes(padrao_f32, **run_kwargs):
    """Run the 8-core Bass kernel; returns m[t] = max_{j<t} cos(p_t, p_j)."""
    global _PROGRAM, _LAST_RESULT
    from concourse.bass_utils import run_bass_kernel_spmd

    if _PROGRAM is None:
        _PROGRAM = _build_program()
    in_maps = _prepare_in_maps(padrao_f32)
    _LAST_RESULT = run_bass_kernel_spmd(
        _PROGRAM, in_maps, list(range(NCORES)), **run_kwargs
    )
    m = np.empty(B, np.float32)
    for k in range(NCORES):
        mo = np.asarray(_LAST_RESULT.results[k]["mout"], dtype=np.float32)
        m[k * 128:(k + 1) * 128] = mo[0]
        m[(15 - k) * 128:(16 - k) * 128] = mo[1]
    return m


# --------------------------------------------------------------------------
# Exact scalar fallback (bit-faithful port of the reference scan)
# --------------------------------------------------------------------------

def _numpy_reference(padrao, erro, prototipos, forca, idade, n_protos):
    Pn, Dn = prototipos.shape
    protos = prototipos.astype(np.float32).copy()
    frc = forca.astype(np.float32).copy()
    age = idade.astype(np.int32).copy()
    n = int(n_protos)
    ar = np.arange(Pn)
    fbs = np.zeros_like(padrao, dtype=np.float32)
    for t in range(padrao.shape[0]):
        p = padrao[t]
        pn = max(np.linalg.norm(p), EPS)
        protn = np.maximum(np.linalg.norm(protos, axis=1), EPS)
        sim = (protos @ p) / (protn * pn)
        sim_m = np.where(ar < n, sim, -np.inf)
        idx = int(np.argmax(sim_m))
        max_sim = sim_m[idx]
        is_empty = n == 0
        do_reinf = (not is_empty) and (max_sim >= LIMIAR_SIM)
        do_create = is_empty or (
            (not do_reinf) and ((erro[t] > LIMIAR_NOVO) or (n < Pn))
        )
        if do_create:
            if n >= Pn:
                cidx = int(np.argmin(np.where(ar < n, frc, np.inf)))
            else:
                cidx = n
            protos[cidx] = p
            frc[cidx] = 1.0
            age[cidx] = 0
            if n < Pn:
                n += 1
            # feedback stays zero
        elif do_reinf:
            new_proto = (1.0 - LR) * protos[idx] + LR * p
            protos[idx] = new_proto
            frc[idx] += LR
            age[idx] += 1
            fbs[t] = (new_proto - p) * frc[idx]
        else:
            fbs[t] = (protos[idx] - p) * frc[idx]
    return fbs


# --------------------------------------------------------------------------
# Entry point
# --------------------------------------------------------------------------

def kernel(padrao, erro, prototipos, forca, idade, n_protos, feedback_gain):
    padrao = np.asarray(padrao, dtype=np.float32)
    erro = np.asarray(erro, dtype=np.float32)
    prototipos = np.asarray(prototipos, dtype=np.float32)
    forca = np.asarray(forca, dtype=np.float32)
    idade = np.asarray(idade, dtype=np.int32)
    n0 = int(np.asarray(n_protos))
    gain = np.float32(np.asarray(feedback_gain))

    fast_ok = (
        padrao.shape == (B, D)
        and prototipos.shape == (P, D)
        and n0 == 0
        and not prototipos.any()
        and not forca.any()
        and not idade.any()
        and B <= P
    )
    if fast_ok:
        m = _device_causal_maxes(padrao)
        # NaN-safe: NaN fails the comparison -> falls back.
        if np.max(m) < GUARD:
            # No step can reinforce (causal max far below 0.7 even after
            # bf16 error); bank starts empty and B <= P, so every step
            # creates and feedback is exactly zero.
            return np.zeros((B, D), np.float32)

    fbs = _numpy_reference(padrao, erro, prototipos, forca, idade, n0)
    return fbs * gain
